# revision 15
# baseline (speedup 1.0000x reference)
"""GAT layer kernel for Trainium2, 8 NeuronCores.

Reference computation:
    X = node_features @ W            [N, DOUT]
    f0 = X @ v0 ; f1 = X @ v1       [N, 1]
    vals = sigmoid(f0 + f1.T) - 0.5
    alphas = softmax(where(graph != 0, vals, -inf), axis=1), masked to 0
    out = elu(alphas @ X)

Key identities used:
  * sigmoid(z) - 0.5 = 0.5*tanh(z/2)  -> tanh and exp live in the same ACT
    table set (one table load instead of per-tile sigmoid<->exp switches).
  * softmax ratio: out_row = (sum_j m_ij e_ij X_j) / (sum_j m_ij e_ij) with
    e = exp(0.5*tanh(z/2)); the row-sum comes free as a ones-column in the
    matmul rhs.
  * The int32 graph matrix viewed as uint16 pairs [lo, hi] (little-endian;
    hi is always 0 because values are 0/1) lets the 2-byte-only DMA xbar
    transpose load graph^T tiles directly from HBM: read the lo-halves with
    element stride 2 and transpose into [128 j, R i] tiles.

Sharding: rows of the graph/output are split across the 8 cores (row-shard,
softmax is row-wise so no cross-core reduction); X is computed per-shard and
AllGathered (as bf16, with f0/f1 folded in as two extra columns of W).
"""

import numpy as np

import concourse.bass as bass
import concourse.mybir as mybir
import concourse.tile as tile
from concourse.bass_utils import run_bass_kernel_spmd

# ----------------------------------------------------------------------------
# Workaround for "Too many sync wait commands": this walrus build accepts only
# ONE sync-wait per instruction. Post-pass: hoist surplus waits onto
# single-wait NOPs on the same engine, inserted immediately before the
# instruction (identical blocking semantics, per-engine order preserved).
# ----------------------------------------------------------------------------


def _split_multi_waits(nc):
    import bass_rust

    eng = {
        mybir.EngineType.PE: nc.tensor,
        mybir.EngineType.DVE: nc.vector,
        mybir.EngineType.Activation: nc.scalar,
        mybir.EngineType.Pool: nc.gpsimd,
        mybir.EngineType.SP: nc.sync,
    }
    for f in nc.m.functions:
        for blk in f.blocks:
            fixups = []  # (index, inst, surplus_waits)
            for idx, inst in enumerate(blk.instructions):
                si = inst.sync_info
                waits = list(si.on_wait) if si is not None and si.on_wait else []
                if len(waits) > 1 and inst.engine in eng:
                    fixups.append((idx, inst, waits))
            if not fixups:
                continue
            nops_by_idx = {}
            created = set()
            for idx, inst, waits in fixups:
                inst.sync_info.on_wait = [waits[-1]]
                nops = []
                for w in waits[:-1]:
                    nop = eng[inst.engine].nop(nofuse=True, hint="wait_split").ins
                    nop.sync_info = bass_rust.SyncInfo(on_wait=[w], on_update=[])
                    nops.append(nop)
                    created.add(id(nop))
                nops_by_idx[idx] = nops
            # Rebuild: drop the freshly-created nops from wherever nop()
            # appended them, then splice them in before their instruction.
            for b2 in f.blocks:
                b2.instructions[:] = [
                    i for i in b2.instructions if id(i) not in created
                ]
            new = []
            for idx, inst in enumerate(blk.instructions):
                new.extend(nops_by_idx.get(idx, ()))
                new.append(inst)
            blk.instructions[:] = new


# ----------------------------------------------------------------------------

F32 = mybir.dt.float32
BF16 = mybir.dt.bfloat16
I32 = mybir.dt.int32
AF = mybir.ActivationFunctionType
ALU = mybir.AluOpType

N, D_IN, D_OUT = 8192, 512, 256
M_CORES = 8
P = 128


def build_gat(n=N, d_in=D_IN, d_out=D_OUT, m_cores=M_CORES, grp=4):
    """Build the per-core SPMD program. Every core runs the same code on its
    row shard: graph rows [R, n] (as a uint16 view [R, 2n]), node-feature
    rows pre-transposed [d_in, R], and the shared W_ext [d_in, d_out+2]
    (= [W | W@v0 | W@v1])."""
    R = n // m_cores            # rows per core
    NJ = n // P                 # 128-wide j chunks over the full N
    IB = R // P                 # 128-row output blocks per core
    DK = d_in // P              # 128-deep contraction chunks for X = nf @ W
    DEXT = d_out + 2            # X | f0 | f1
    DW = d_out + 1              # matmul rhs width: X | ones
    n_grp = NJ // grp           # j-chunk groups (batch ACT/DVE work)

    nc = bass.Bass(num_devices=m_cores)
    g_t = nc.declare_dram_parameter("graph_T", [n, R], I32, isOutput=False)
    nfT = nc.declare_dram_parameter("nfT", [d_in, R], F32, isOutput=False)
    wext = nc.declare_dram_parameter("wext", [d_in, DEXT], F32, isOutput=False)
    outp = nc.declare_dram_parameter("out", [R, d_out], F32, isOutput=True)

    with tile.TileContext(nc) as tc:
        with tc.tile_pool(name="persist", bufs=1) as persist, \
             tc.tile_pool(name="dram", bufs=1, space="DRAM") as dram:
            xsb = persist.tile([P, NJ, DEXT], BF16)      # gathered X~ (bf16)
            f0rep = persist.tile([P, R], F32)            # f0 row replicated
            f1half = persist.tile([P, NJ], F32)          # 0.5*f1 per partition

            # ---------------- Stage A: X~ = nf @ W_ext, f0/f1, AllGather ----
            with tc.tile_pool(name="stageA", bufs=1) as sa, \
                 tc.tile_pool(name="psumA", bufs=2, space="PSUM") as psa:
                nfTsb = sa.tile([P, DK, R], F32)
                nc.sync.dma_start(
                    out=nfTsb,
                    in_=bass.AP(nfT, 0, [[R, P], [P * R, DK], [1, R]]),
                )
                wsb = sa.tile([P, DK, DEXT], F32)
                nc.sync.dma_start(
                    out=wsb,
                    in_=bass.AP(wext, 0, [[DEXT, P], [P * DEXT, DK], [1, DEXT]]),
                )
                xc = sa.tile([P, IB, DEXT], F32)
                xcb = sa.tile([P, IB, DEXT], BF16)
                for ib in range(IB):
                    psx = psa.tile([P, DEXT], F32, tag="psx")
                    for kc in range(DK):
                        nc.tensor.matmul(
                            out=psx,
                            lhsT=nfTsb[:, kc, ib * P:(ib + 1) * P],
                            rhs=wsb[:, kc, :],
                            start=(kc == 0),
                            stop=(kc == DK - 1),
                        )
                    nc.vector.tensor_copy(out=xc[:, ib, :], in_=psx)
                    nc.vector.tensor_copy(out=xcb[:, ib, :], in_=xc[:, ib, :])

                # f0 column -> DRAM (linear over this core's rows)
                f0dram = dram.tile([R], F32)
                nc.gpsimd.dma_start(
                    out=f0dram.rearrange("(ib p) -> p ib", p=P),
                    in_=xc[:, :, d_out],
                )
                # broadcast back across partitions
                nc.sync.dma_start(
                    out=f0rep,
                    in_=bass.AP(f0dram.tensor, 0, [[0, P], [1, R]]),
                )

                # AllGather X~ (bf16) across cores via DRAM bounce
                ag_in = dram.tile([R, DEXT], BF16)
                nc.gpsimd.dma_start(
                    out=ag_in.rearrange("(ib p) d -> p ib d", p=P),
                    in_=xcb,
                )
                ag_out = dram.tile(
                    [n, DEXT], BF16,
                    addr_space="Shared" if m_cores > 4 else "Local",
                )
                nc.gpsimd.collective_compute(
                    "AllGather",
                    ALU.bypass,
                    replica_groups=[list(range(m_cores))],
                    ins=[ag_in.opt()],
                    outs=[ag_out.opt()],
                )
                nc.sync.dma_start(
                    out=xsb,
                    in_=bass.AP(
                        ag_out.tensor, 0, [[DEXT, P], [P * DEXT, NJ], [1, DEXT]]
                    ),
                )

            # f1 per partition (0.5x, fp32) then ones column over f0 slot
            nc.vector.tensor_scalar_mul(f1half, xsb[:, :, d_out + 1], 0.5)
            nc.vector.memset(xsb[:, :, d_out], 1.0)

            # ---------------- Stage B: masked-softmax matmul ----------------
            with tc.tile_pool(name="psumB", bufs=1, space="PSUM") as psb, \
                 tc.tile_pool(name="mask", bufs=2) as mask_pool, \
                 tc.tile_pool(name="tg", bufs=2) as t_pool, \
                 tc.tile_pool(name="eg", bufs=2) as e_pool, \
                 tc.tile_pool(name="pg", bufs=2) as p_pool, \
                 tc.tile_pool(name="epi", bufs=2) as epi:
                psum = [
                    psb.tile([P, DW], F32, tag=f"ps{ib}", name=f"psum{ib}")
                    for ib in range(IB)
                ]
                for g in range(n_grp):
                    m32 = mask_pool.tile([P, grp, R], I32)
                    t_g = t_pool.tile([P, grp, R], BF16)
                    for jj in range(grp):
                        jc = g * grp + jj
                        # graph^T rows for this j chunk: [128 j, R i]
                        nc.sync.dma_start(
                            out=m32[:, jj, :],
                            in_=g_t[jc * P:(jc + 1) * P, :],
                        )
                        nc.scalar.activation(
                            out=t_g[:, jj, :],
                            in_=f0rep,
                            func=AF.Tanh,
                            bias=f1half[:, jc:jc + 1],
                            scale=0.5,
                        )
                    e_g = e_pool.tile([P, grp, R], BF16)
                    nc.scalar.activation(out=e_g, in_=t_g, func=AF.Exp, scale=0.5)
                    p_g = p_pool.tile([P, grp, R], BF16)
                    nc.vector.tensor_tensor(
                        out=p_g, in0=m32, in1=e_g, op=ALU.mult
                    )
                    for jj in range(grp):
                        jc = g * grp + jj
                        for ib in range(IB):
                            nc.tensor.matmul(
                                out=psum[ib],
                                lhsT=p_g[:, jj, ib * P:(ib + 1) * P],
                                rhs=xsb[:, jc, 0:DW],
                                start=(jc == 0),
                                stop=(jc == NJ - 1),
                            )

                # ---------------- Epilogue: normalize + elu + store --------
                for ib in range(IB):
                    o = epi.tile([P, DW], F32, tag="o")
                    nc.vector.tensor_copy(out=o, in_=psum[ib])
                    sm = epi.tile([P, 1], F32, tag="sm")
                    nc.vector.tensor_scalar_max(sm, o[:, d_out:DW], 1e-30)
                    r = epi.tile([P, 1], F32, tag="r")
                    nc.vector.reciprocal(out=r, in_=sm)
                    u = epi.tile([P, d_out], F32, tag="u")
                    nc.vector.tensor_scalar(
                        out=u, in0=o[:, 0:d_out], scalar1=r, scalar2=None,
                        op0=ALU.mult,
                    )
                    rp = epi.tile([P, d_out], F32, tag="rp")
                    nc.vector.tensor_scalar_max(rp, u, 0.0)
                    xm = epi.tile([P, d_out], F32, tag="xm")
                    nc.vector.tensor_scalar_min(xm, u, 0.0)
                    en = epi.tile([P, d_out], F32, tag="en")
                    nc.scalar.activation(out=en, in_=xm, func=AF.Exp)
                    res = epi.tile([P, d_out], F32, tag="res")
                    nc.vector.tensor_tensor(out=res, in0=en, in1=rp, op=ALU.add)
                    nc.vector.tensor_scalar_add(res, res, -1.0)
                    nc.sync.dma_start(
                        out=outp[ib * P:(ib + 1) * P, :], in_=res
                    )
    _split_multi_waits(nc)
    return nc


_cached = {}

# Dev/test knobs (the grading harness just calls kernel(**inputs)):
_TRACE = False
_TMPDIR = None
_LAST_EXEC_NS = None
_LAST_RESULTS = None


def _get_program(n, d_in, d_out, m_cores):
    key = (n, d_in, d_out, m_cores)
    if key not in _cached:
        _cached[key] = build_gat(n, d_in, d_out, m_cores)
    return _cached[key]


def kernel(node_features, graph, W, v0, v1):
    node_features = np.asarray(node_features, dtype=np.float32)
    graph = np.ascontiguousarray(np.asarray(graph, dtype=np.int32))
    W = np.asarray(W, dtype=np.float32)
    v0 = np.asarray(v0, dtype=np.float32)
    v1 = np.asarray(v1, dtype=np.float32)

    n, d_in = node_features.shape
    d_out = W.shape[1]
    m = M_CORES
    R = n // m

    nc = _get_program(n, d_in, d_out, m)

    wext = np.concatenate([W, W @ v0, W @ v1], axis=1).astype(np.float32)
    in_maps = []
    for c in range(m):
        rows = slice(c * R, (c + 1) * R)
        in_maps.append({
            "graph_T": np.ascontiguousarray(graph[rows].T),
            "nfT": np.ascontiguousarray(node_features[rows].T),
            "wext": wext,
        })
    global _LAST_EXEC_NS, _LAST_RESULTS
    res = run_bass_kernel_spmd(
        nc, in_maps, list(range(m)), trace=_TRACE, tmpdir=_TMPDIR
    )
    _LAST_EXEC_NS = res.exec_time_ns
    _LAST_RESULTS = res
    return np.concatenate([res.results[c]["out"] for c in range(m)], axis=0)
